# revision 8
# baseline (speedup 1.0000x reference)
"""MoE layer (8 experts, top-2) on 8 Trainium2 NeuronCores.

Strategy (expert-parallel, per the sharding hint):
  - Host computes the router (logits -> softmax -> top-2) and plays the role
    of the all-to-all: tokens are gathered per expert, transposed and padded
    to a common capacity C, and each of the 8 cores is handed exactly one
    expert's token batch plus that expert's weights.
  - Each core runs the expert FFN  yT = W2.T @ relu(W1.T @ xT + b1) + b2
    entirely on-device with fp32r matmuls (full PE rate at free-dim >= 256),
    tiled over 512-token chunks with a one-chunk software pipeline.
  - Host applies the gate weights during the scatter-add combine and computes
    the (tiny) aux loss.

The kernel is self-contained: shapes/sharding are hardcoded for the
B=4, T=2048, D=512, H=2048, E=8, top-2 problem, but C (per-expert capacity)
is derived from the actual routing of the inputs at call time.
"""

import os
import sys

import numpy as np

for _p in ("/opt/trn_rl_repo", "/root/.axon_site/_ro/trn_rl_repo"):
    if _p not in sys.path and os.path.isdir(_p):
        sys.path.append(_p)

import concourse.bass as bass  # noqa: E402
import concourse.mybir as mybir  # noqa: E402
from concourse import bacc, tile  # noqa: E402
from concourse.bass_utils import run_bass_kernel_spmd  # noqa: E402

E, D, H, TOPK = 8, 512, 2048, 2
B, T = 4, 2048
NT = B * T  # 8192 tokens
P = 128
KO = D // P   # 4  k-chunks for mm1 (contract over D)
MO = H // P   # 16 h-chunks (mm1 output partitions / mm2 contraction)
DO = D // P   # 4  d-chunks for mm2 output
NCHUNK = 512  # token chunk (matmul moving free dim)

# Results of the last device run (for the test harness: exec_time_ns etc.)
last_run = None
last_exec_wall = None
_build_cache = {}


def _build(C):
    """Build + compile the single-expert FFN SPMD program for capacity C."""
    f32 = mybir.dt.float32
    f32r = mybir.dt.float32r
    add = mybir.AluOpType.add
    maxop = mybir.AluOpType.max

    nc = bacc.Bacc("TRN2", target_bir_lowering=False, debug=False, num_devices=E)

    xt_d = nc.dram_tensor("xt", [D, C], f32r, kind="ExternalInput")
    w1_d = nc.dram_tensor("w1", [D, H], f32r, kind="ExternalInput")
    w2_d = nc.dram_tensor("w2", [H, D], f32r, kind="ExternalInput")
    b1_d = nc.dram_tensor("b1", [H], f32, kind="ExternalInput")
    b2_d = nc.dram_tensor("b2", [D], f32, kind="ExternalInput")
    yt_d = nc.dram_tensor("yt", [D, C], f32, kind="ExternalOutput")

    # token chunks: full 512s plus a multiple-of-128 tail
    chunks = []
    off = 0
    while off < C:
        cn = min(NCHUNK, C - off)
        chunks.append((off, cn))
        off += cn
    nchunks = len(chunks)

    yt_v = yt_d.ap().rearrange("(do p) c -> p do c", p=P)

    xt_v = xt_d.ap().rearrange("(ko p) c -> p ko c", p=P)
    w1_v = w1_d.ap().rearrange("(ko p) h -> p ko h", p=P)
    w2_v = w2_d.ap().rearrange("(mo p) d -> p mo d", p=P)

    with tile.TileContext(nc) as tc:
        with (
            tc.tile_pool(name="wts", bufs=1) as wpool,
            tc.tile_pool(name="xt", bufs=1) as xpool,
            tc.tile_pool(name="h", bufs=2) as hpool,
            tc.tile_pool(name="o", bufs=4) as opool,
            tc.tile_pool(name="ps1", bufs=2, space="PSUM") as ps1pool,
            tc.tile_pool(name="ps2", bufs=6, space="PSUM") as ps2pool,
        ):
            # DMAs are split into pieces and emitted in first-use order so the
            # PE starts a few us in instead of waiting ~40us for all inputs.
            b1_sb = wpool.tile([P, MO], f32)
            b2_sb = wpool.tile([P, DO], f32)
            xt_sb = xpool.tile([P, KO, C], f32r)
            w1_sb = wpool.tile([P, KO, H], f32r)
            w2_sb = wpool.tile([P, MO, D], f32r)

            def load_xt(ci):
                off, cn = chunks[ci]
                if ci == 0:
                    for k in range(KO):  # finer pieces: first matmul needs k=0 only
                        nc.sync.dma_start(
                            xt_sb[:, k, off : off + cn], xt_v[:, k, off : off + cn]
                        )
                else:
                    nc.sync.dma_start(
                        xt_sb[:, :, off : off + cn], xt_v[:, :, off : off + cn]
                    )

            def load_w1(m):
                nc.sync.dma_start(
                    w1_sb[:, :, m * P : (m + 1) * P], w1_v[:, :, m * P : (m + 1) * P]
                )

            # the very first matmul needs only xt0[k=0] and w1[m=0]
            off0, cn0 = chunks[0]
            nc.sync.dma_start(xt_sb[:, 0, off0 : off0 + cn0], xt_v[:, 0, off0 : off0 + cn0])
            load_w1(0)
            for k in range(1, KO):
                nc.sync.dma_start(
                    xt_sb[:, k, off0 : off0 + cn0], xt_v[:, k, off0 : off0 + cn0]
                )
            nc.sync.dma_start(b1_sb[:], b1_d.ap().rearrange("(mo p) -> p mo", p=P))
            nc.sync.dma_start(b2_sb[:], b2_d.ap().rearrange("(do p) -> p do", p=P))
            for m in range(1, MO):
                load_w1(m)
            if nchunks > 1:
                load_xt(1)
            for m in range(MO):
                nc.sync.dma_start(w2_sb[:, m], w2_v[:, m])
            for ci in range(2, nchunks):
                load_xt(ci)

            h_tiles = [None] * nchunks

            def mm1(ci):
                off, cn = chunks[ci]
                h_sb = hpool.tile([P, MO, NCHUNK], f32r)
                h_tiles[ci] = h_sb
                for m in range(MO):
                    ps = ps1pool.tile([P, NCHUNK], f32)
                    for k in range(KO):
                        nc.tensor.matmul(
                            ps[:, :cn],
                            w1_sb[:, k, m * P : (m + 1) * P],
                            xt_sb[:, k, off : off + cn],
                            start=(k == 0),
                            stop=(k == KO - 1),
                        )
                    # h = relu(ps + b1)  (DVE: out = max(ps + b1, 0))
                    nc.vector.tensor_scalar(
                        h_sb[:, m, :cn], ps[:, :cn], b1_sb[:, m : m + 1], 0.0, add, maxop
                    )

            def drain_ps2(ps2_t, dd, off, cn):
                o_sb = opool.tile([P, NCHUNK], f32, name=f"o_{off}_{dd}", tag="o")
                nc.vector.tensor_scalar(
                    o_sb[:, :cn], ps2_t[:, :cn], b2_sb[:, dd : dd + 1], None, add
                )
                nc.sync.dma_start(yt_v[:, dd, off : off + cn], o_sb[:, :cn])

            def mm2(ci):
                off, cn = chunks[ci]
                h_sb = h_tiles[ci]
                if ci < nchunks - 1:
                    # m-outer / dd-inner: w2 is consumed one m-piece at a time,
                    # so mm2(0) can start as soon as the first w2 piece lands.
                    ps2s = []
                    for _dd in range(DO):
                        ps2_t = ps2pool.tile(
                            [P, NCHUNK], f32, name=f"ps2_{ci}_{_dd}", tag="ps2"
                        )
                        ps2s.append(ps2_t)
                    for m in range(MO):
                        for dd in range(DO):
                            nc.tensor.matmul(
                                ps2s[dd][:, :cn],
                                w2_sb[:, m, dd * P : (dd + 1) * P],
                                h_sb[:, m, :cn],
                                start=(m == 0),
                                stop=(m == MO - 1),
                            )
                    for dd in range(DO):
                        drain_ps2(ps2s[dd], dd, off, cn)
                else:
                    # last chunk: dd-outer so each PSUM bank completes early and
                    # its bias-add + store drain while the PE finishes the rest.
                    for dd in range(DO):
                        ps2_t = ps2pool.tile(
                            [P, NCHUNK], f32, name=f"ps2_{ci}_{dd}", tag="ps2"
                        )
                        for m in range(MO):
                            nc.tensor.matmul(
                                ps2_t[:, :cn],
                                w2_sb[:, m, dd * P : (dd + 1) * P],
                                h_sb[:, m, :cn],
                                start=(m == 0),
                                stop=(m == MO - 1),
                            )
                        drain_ps2(ps2_t, dd, off, cn)

            # one-chunk software pipeline: mm1(ci) runs on PE while the relu of
            # chunk ci-1 finishes, so mm2(ci-1) never stalls the PE.
            for ci in range(nchunks + 1):
                if ci < nchunks:
                    mm1(ci)
                if ci >= 1:
                    mm2(ci - 1)

    nc.compile()
    return nc


def kernel(X, router_w, w1, b1, w2, b2):
    global last_run
    X = np.ascontiguousarray(np.asarray(X, dtype=np.float32))
    router_w = np.ascontiguousarray(np.asarray(router_w, dtype=np.float32))
    w1 = np.ascontiguousarray(np.asarray(w1, dtype=np.float32))
    b1 = np.ascontiguousarray(np.asarray(b1, dtype=np.float32))
    w2 = np.ascontiguousarray(np.asarray(w2, dtype=np.float32))
    b2 = np.ascontiguousarray(np.asarray(b2, dtype=np.float32))

    Xf = X.reshape(NT, D)

    # --- router on host (fp32, mirroring the jax reference) ---
    logits = Xf @ router_w                       # [NT, E]
    m = logits.max(axis=-1, keepdims=True)
    ex = np.exp(logits - m)
    probs = ex / ex.sum(axis=-1, keepdims=True)  # [NT, E]
    order = np.argsort(-probs, axis=-1, kind="stable")
    idx = order[:, :TOPK]                        # [NT, K] expert ids
    wts = np.take_along_axis(probs, idx, axis=-1)  # [NT, K]

    # --- dispatch: gather tokens per expert ---
    ids = []
    gates = []
    counts = np.zeros(E, dtype=np.int64)
    for e in range(E):
        mask = idx == e                          # [NT, K]
        rows = mask.any(axis=1)
        ids_e = np.nonzero(rows)[0]
        g_e = (wts * mask).sum(axis=1)[ids_e].astype(np.float32)
        ids.append(ids_e)
        gates.append(g_e)
        counts[e] = len(ids_e)

    C = max(NCHUNK, int(-(-counts.max() // P) * P))  # round up to 128, >= 512

    in_maps = []
    for e in range(E):
        xt = np.zeros((D, C), dtype=np.float32)
        ne = counts[e]
        if ne:
            xt[:, :ne] = Xf[ids[e]].T
        in_maps.append(
            {
                "xt": xt,
                "w1": w1[e],
                "w2": w2[e],
                "b1": b1[e],
                "b2": b2[e],
            }
        )

    if C not in _build_cache:
        _build_cache[C] = _build(C)
    nc = _build_cache[C]

    import time as _time

    _t0 = _time.time()
    last_run = run_bass_kernel_spmd(nc, in_maps, core_ids=list(range(E)))
    global last_exec_wall
    last_exec_wall = _time.time() - _t0

    # --- combine (host applies gate weights; ids within an expert are unique) ---
    out = np.zeros((NT, D), dtype=np.float32)
    for e in range(E):
        ne = counts[e]
        if ne:
            yt = last_run.results[e]["yt"]       # [D, C]
            out[ids[e]] += yt[:, :ne].T * gates[e][:, None]

    output = out.reshape(B, T, D)

    # --- aux loss (host, fp32 like the reference) ---
    density = counts.astype(np.float32) / np.float32(NT)
    importance = np.array(
        [g.sum(dtype=np.float32) for g in gates], dtype=np.float32
    ) / np.float32(NT)
    aux_loss = np.float32(E) * np.sum(density * importance, dtype=np.float32)

    return output, np.float32(aux_loss)


# revision 12
# speedup vs baseline: 1.1079x; 1.1079x over previous
"""MoE layer (8 experts, top-2) on 8 Trainium2 NeuronCores.

Strategy (expert-parallel, per the sharding hint):
  - Host computes the router (logits -> softmax -> top-2) and plays the role
    of the all-to-all: tokens are gathered per expert, transposed and padded
    to a common capacity C, and each of the 8 cores is handed exactly one
    expert's token batch plus that expert's weights.
  - Each core runs the expert FFN  yT = W2.T @ relu(W1.T @ xT + b1) + b2
    entirely on-device with fp32r matmuls (full PE rate at free-dim >= 256),
    tiled over 512-token chunks with a one-chunk software pipeline.
  - Host applies the gate weights during the scatter-add combine and computes
    the (tiny) aux loss.

The kernel is self-contained: shapes/sharding are hardcoded for the
B=4, T=2048, D=512, H=2048, E=8, top-2 problem, but C (per-expert capacity)
is derived from the actual routing of the inputs at call time.
"""

import os
import sys

import numpy as np

for _p in ("/opt/trn_rl_repo", "/root/.axon_site/_ro/trn_rl_repo"):
    if _p not in sys.path and os.path.isdir(_p):
        sys.path.append(_p)

import concourse.bass as bass  # noqa: E402
import concourse.mybir as mybir  # noqa: E402
from concourse import bacc, tile  # noqa: E402
from concourse.bass_utils import run_bass_kernel_spmd  # noqa: E402

E, D, H, TOPK = 8, 512, 2048, 2
B, T = 4, 2048
NT = B * T  # 8192 tokens
P = 128
KO = D // P   # 4  k-chunks for mm1 (contract over D)
MO = H // P   # 16 h-chunks (mm1 output partitions / mm2 contraction)
DO = D // P   # 4  d-chunks for mm2 output
NCHUNK = 512  # token chunk (matmul moving free dim)

# Results of the last device run (for the test harness: exec_time_ns etc.)
last_run = None
last_exec_wall = None
_build_cache = {}


def _build(C):
    """Build + compile the single-expert FFN SPMD program for capacity C."""
    f32 = mybir.dt.float32
    f32r = mybir.dt.float32r
    add = mybir.AluOpType.add
    maxop = mybir.AluOpType.max

    nc = bacc.Bacc("TRN2", target_bir_lowering=False, debug=False, num_devices=E)

    xt_d = nc.dram_tensor("xt", [D, C], f32r, kind="ExternalInput")
    w1_d = nc.dram_tensor("w1", [D, H], f32r, kind="ExternalInput")
    w2_d = nc.dram_tensor("w2", [H, D], f32r, kind="ExternalInput")
    b1_d = nc.dram_tensor("b1", [H], f32, kind="ExternalInput")
    b2_d = nc.dram_tensor("b2", [D], f32, kind="ExternalInput")
    yt_d = nc.dram_tensor("yt", [D, C], f32, kind="ExternalOutput")

    # token chunks: a short 256 head chunk (so the PE can start as soon as the
    # first DMA pieces land), then full 512s, then a multiple-of-128 tail
    chunks = []
    off = 0
    if C >= 768:
        chunks.append((0, 256))
        off = 256
    while off < C:
        cn = min(NCHUNK, C - off)
        chunks.append((off, cn))
        off += cn
    nchunks = len(chunks)

    yt_v = yt_d.ap().rearrange("(do p) c -> p do c", p=P)

    xt_v = xt_d.ap().rearrange("(ko p) c -> p ko c", p=P)
    w1_v = w1_d.ap().rearrange("(ko p) h -> p ko h", p=P)
    w2_v = w2_d.ap().rearrange("(mo p) d -> p mo d", p=P)

    with tile.TileContext(nc) as tc:
        with (
            tc.tile_pool(name="wts", bufs=1) as wpool,
            tc.tile_pool(name="xt", bufs=1) as xpool,
            tc.tile_pool(name="h", bufs=2) as hpool,
            tc.tile_pool(name="o", bufs=4) as opool,
            tc.tile_pool(name="ps1", bufs=2, space="PSUM") as ps1pool,
            tc.tile_pool(name="ps2", bufs=6, space="PSUM") as ps2pool,
        ):
            # DMAs are split into pieces and emitted in first-use order so the
            # PE starts a few us in instead of waiting ~40us for all inputs.
            b1_sb = wpool.tile([P, MO], f32)
            b2_sb = wpool.tile([P, DO], f32)
            xt_sb = xpool.tile([P, KO, C], f32r)
            w1_sb = wpool.tile([P, KO, H], f32r)
            w2_sb = wpool.tile([P, MO, D], f32r)

            def load_xt(ci):
                off, cn = chunks[ci]
                if ci == 0:
                    for k in range(KO):  # finer pieces: first matmul needs k=0 only
                        nc.sync.dma_start(
                            xt_sb[:, k, off : off + cn], xt_v[:, k, off : off + cn]
                        )
                else:
                    nc.sync.dma_start(
                        xt_sb[:, :, off : off + cn], xt_v[:, :, off : off + cn]
                    )

            def load_w1(m):
                nc.sync.dma_start(
                    w1_sb[:, :, m * P : (m + 1) * P], w1_v[:, :, m * P : (m + 1) * P]
                )

            # the very first matmul needs only xt0[k=0] and w1[m=0]
            off0, cn0 = chunks[0]
            nc.sync.dma_start(xt_sb[:, 0, off0 : off0 + cn0], xt_v[:, 0, off0 : off0 + cn0])
            load_w1(0)
            for k in range(1, KO):
                nc.sync.dma_start(
                    xt_sb[:, k, off0 : off0 + cn0], xt_v[:, k, off0 : off0 + cn0]
                )
            nc.sync.dma_start(b1_sb[:], b1_d.ap().rearrange("(mo p) -> p mo", p=P))
            nc.sync.dma_start(b2_sb[:], b2_d.ap().rearrange("(do p) -> p do", p=P))
            for m in range(1, MO):
                load_w1(m)
            if nchunks > 1:
                load_xt(1)
            for m in range(MO):
                nc.sync.dma_start(w2_sb[:, m], w2_v[:, m])
            for ci in range(2, nchunks):
                load_xt(ci)

            h_tiles = [None] * nchunks

            def mm1(ci):
                off, cn = chunks[ci]
                h_sb = hpool.tile([P, MO, NCHUNK], f32r)
                h_tiles[ci] = h_sb
                for m in range(MO):
                    ps = ps1pool.tile([P, NCHUNK], f32)
                    for k in range(KO):
                        nc.tensor.matmul(
                            ps[:, :cn],
                            w1_sb[:, k, m * P : (m + 1) * P],
                            xt_sb[:, k, off : off + cn],
                            start=(k == 0),
                            stop=(k == KO - 1),
                        )
                    # h = relu(ps + b1)  (DVE: out = max(ps + b1, 0))
                    nc.vector.tensor_scalar(
                        h_sb[:, m, :cn], ps[:, :cn], b1_sb[:, m : m + 1], 0.0, add, maxop
                    )

            def drain_ps2(ps2_t, dd, off, cn):
                o_sb = opool.tile([P, NCHUNK], f32, name=f"o_{off}_{dd}", tag="o")
                nc.vector.tensor_scalar(
                    o_sb[:, :cn], ps2_t[:, :cn], b2_sb[:, dd : dd + 1], None, add
                )
                nc.sync.dma_start(yt_v[:, dd, off : off + cn], o_sb[:, :cn])

            def mm2(ci):
                off, cn = chunks[ci]
                h_sb = h_tiles[ci]
                if ci < nchunks - 1:
                    # m-outer / dd-inner: w2 is consumed one m-piece at a time,
                    # so mm2(0) can start as soon as the first w2 piece lands.
                    ps2s = []
                    for _dd in range(DO):
                        ps2_t = ps2pool.tile(
                            [P, NCHUNK], f32, name=f"ps2_{ci}_{_dd}", tag="ps2"
                        )
                        ps2s.append(ps2_t)
                    for m in range(MO):
                        for dd in range(DO):
                            nc.tensor.matmul(
                                ps2s[dd][:, :cn],
                                w2_sb[:, m, dd * P : (dd + 1) * P],
                                h_sb[:, m, :cn],
                                start=(m == 0),
                                stop=(m == MO - 1),
                            )
                    for dd in range(DO):
                        drain_ps2(ps2s[dd], dd, off, cn)
                else:
                    # last chunk: dd-outer so each PSUM bank completes early and
                    # its bias-add + store drain while the PE finishes the rest.
                    for dd in range(DO):
                        ps2_t = ps2pool.tile(
                            [P, NCHUNK], f32, name=f"ps2_{ci}_{dd}", tag="ps2"
                        )
                        for m in range(MO):
                            nc.tensor.matmul(
                                ps2_t[:, :cn],
                                w2_sb[:, m, dd * P : (dd + 1) * P],
                                h_sb[:, m, :cn],
                                start=(m == 0),
                                stop=(m == MO - 1),
                            )
                        drain_ps2(ps2_t, dd, off, cn)

            # one-chunk software pipeline: mm1(ci) runs on PE while the relu of
            # chunk ci-1 finishes, so mm2(ci-1) never stalls the PE.
            for ci in range(nchunks + 1):
                if ci < nchunks:
                    mm1(ci)
                if ci >= 1:
                    mm2(ci - 1)

    nc.compile()
    return nc


def kernel(X, router_w, w1, b1, w2, b2):
    global last_run
    X = np.ascontiguousarray(np.asarray(X, dtype=np.float32))
    router_w = np.ascontiguousarray(np.asarray(router_w, dtype=np.float32))
    w1 = np.ascontiguousarray(np.asarray(w1, dtype=np.float32))
    b1 = np.ascontiguousarray(np.asarray(b1, dtype=np.float32))
    w2 = np.ascontiguousarray(np.asarray(w2, dtype=np.float32))
    b2 = np.ascontiguousarray(np.asarray(b2, dtype=np.float32))

    Xf = X.reshape(NT, D)

    # --- router on host (fp32, mirroring the jax reference) ---
    logits = Xf @ router_w                       # [NT, E]
    m = logits.max(axis=-1, keepdims=True)
    ex = np.exp(logits - m)
    probs = ex / ex.sum(axis=-1, keepdims=True)  # [NT, E]
    order = np.argsort(-probs, axis=-1, kind="stable")
    idx = order[:, :TOPK]                        # [NT, K] expert ids
    wts = np.take_along_axis(probs, idx, axis=-1)  # [NT, K]

    # --- dispatch: gather tokens per expert ---
    ids = []
    gates = []
    counts = np.zeros(E, dtype=np.int64)
    for e in range(E):
        mask = idx == e                          # [NT, K]
        rows = mask.any(axis=1)
        ids_e = np.nonzero(rows)[0]
        g_e = (wts * mask).sum(axis=1)[ids_e].astype(np.float32)
        ids.append(ids_e)
        gates.append(g_e)
        counts[e] = len(ids_e)

    C = max(NCHUNK, int(-(-counts.max() // P) * P))  # round up to 128, >= 512

    in_maps = []
    for e in range(E):
        xt = np.zeros((D, C), dtype=np.float32)
        ne = counts[e]
        if ne:
            xt[:, :ne] = Xf[ids[e]].T
        in_maps.append(
            {
                "xt": xt,
                "w1": w1[e],
                "w2": w2[e],
                "b1": b1[e],
                "b2": b2[e],
            }
        )

    if C not in _build_cache:
        _build_cache[C] = _build(C)
    nc = _build_cache[C]

    import time as _time

    _t0 = _time.time()
    try:
        last_run = run_bass_kernel_spmd(nc, in_maps, core_ids=list(range(E)))
    except ModuleNotFoundError:
        # BASS_TRACE was requested but this axon build has no NTFF hook
        os.environ["BASS_NEVER_TRACE"] = "1"
        last_run = run_bass_kernel_spmd(nc, in_maps, core_ids=list(range(E)))
    global last_exec_wall
    last_exec_wall = _time.time() - _t0

    # --- combine (host applies gate weights; ids within an expert are unique) ---
    out = np.zeros((NT, D), dtype=np.float32)
    for e in range(E):
        ne = counts[e]
        if ne:
            yt = last_run.results[e]["yt"]       # [D, C]
            out[ids[e]] += yt[:, :ne].T * gates[e][:, None]

    output = out.reshape(B, T, D)

    # --- aux loss (host, fp32 like the reference) ---
    density = counts.astype(np.float32) / np.float32(NT)
    importance = np.array(
        [g.sum(dtype=np.float32) for g in gates], dtype=np.float32
    ) / np.float32(NT)
    aux_loss = np.float32(E) * np.sum(density * importance, dtype=np.float32)

    return output, np.float32(aux_loss)
